# revision 1
# baseline (speedup 1.0000x reference)
"""TRN2 Bass kernel for nn_Construct_76484777607483.

Computes, for 12 input tensors x_i [B=2, C=256, H=64, W=256]:
    y_i = einsum('bchw,co->bohw', x_i, W)
interleaved over H (output row 12*h + i comes from tensor i, row h) into
out [2, 256, 768, 256], plus bias b[o] * count(row) where count is the
conv-transpose overlap multiplicity (ramp 1..12 at the top edge, 12 in the
middle, 12..1 at the bottom edge).

Sharding: 8 cores = (2 batches) x (4 h-quarters of 16 input rows). Each core
handles all 12 tensors for its 16 rows, so the row-interleave is assembled
on-chip and output DMA writes are fully contiguous per channel.

Per-core kernel: for each group of 2 input rows (512 pixels), for each tensor
i, a [256 -> 256] channel matmul is done as 2 accumulating 128x128x512
matmuls in float32r (full-rate PE path, ~1.5e-4 rel err), then the PSUM tile
is copied into an interleave-layout SBUF buffer with the per-(i, h) bias
value added as a per-partition scalar (DVE tensor_scalar_add). The bias
values (b[o] * count) are precomputed on host per core.
"""

import numpy as np

import concourse.bacc as bacc
import concourse.tile as tile
import concourse.mybir as mybir
from concourse.bass_utils import run_bass_kernel_spmd

B, C, H, WD = 2, 256, 64, 256
NT = 12                 # stacked tensors
NCORES = 8
HQ = H // 4             # 16 input rows per core
NG = HQ // 2            # 8 groups of 2 rows
HOUT = NT * H           # 768

_F32 = mybir.dt.float32
_F32R = mybir.dt.float32r

_NC_CACHE = {}


def build_nc():
    if "nc" in _NC_CACHE:
        return _NC_CACHE["nc"]
    nc = bacc.Bacc("TRN2", target_bir_lowering=False)
    x_d = nc.declare_dram_parameter("x", [NT, C, HQ, WD], _F32R, isOutput=False)
    w_d = nc.declare_dram_parameter("w", [C, C], _F32R, isOutput=False)
    bv_d = nc.declare_dram_parameter("bv", [2, 128, NT * HQ], _F32, isOutput=False)
    y_d = nc.declare_dram_parameter("y", [C, NT * HQ, WD], _F32, isOutput=True)

    with tile.TileContext(nc) as tc:
        with (
            tc.tile_pool(name="const", bufs=1) as cpool,
            tc.tile_pool(name="xin", bufs=6) as inpool,
            tc.tile_pool(name="obuf", bufs=3) as outpool,
            tc.tile_pool(name="ps", bufs=4, space="PSUM") as pspool,
        ):
            wt = [
                [
                    cpool.tile([128, 128], _F32R, name=f"w{kh}{mh}")
                    for mh in range(2)
                ]
                for kh in range(2)
            ]
            for kh in range(2):
                for mh in range(2):
                    # consts load on the ACT ring so the SP ring starts the
                    # first input tiles immediately
                    nc.scalar.dma_start(
                        out=wt[kh][mh][:],
                        in_=w_d[kh * 128 : (kh + 1) * 128, mh * 128 : (mh + 1) * 128],
                    )
            bvt = [cpool.tile([128, NT * HQ], _F32, name=f"bv{mh}") for mh in range(2)]
            for mh in range(2):
                nc.scalar.dma_start(out=bvt[mh][:], in_=bv_d[mh])

            for g in range(NG):
                obufs = [
                    outpool.tile(
                        [128, 2, NT, WD], _F32, name=f"ob{g}_{mh}", tag=f"ob{mh}"
                    )
                    for mh in range(2)
                ]
                for i0 in range(0, NT, 2):
                    xps = []
                    for i in (i0, i0 + 1):
                        xin = inpool.tile(
                            [128, 2, 2, WD], _F32R, name=f"xin{g}_{i}", tag="xin"
                        )
                        for kh in range(2):
                            eng = nc.gpsimd if (kh == 1 and i >= 5) else nc.sync
                            eng.dma_start(
                                out=xin[:, kh],
                                in_=x_d[
                                    i, kh * 128 : (kh + 1) * 128, 2 * g : 2 * g + 2, :
                                ],
                            )
                        xps.append(xin)
                    for mh in range(2):
                        # one 2-bank PSUM tile per tensor PAIR [128, ip, hl, WD]
                        ps = pspool.tile(
                            [128, 2, 2, WD], _F32, name=f"ps{g}_{i0}_{mh}", tag="ps"
                        )
                        for ip in range(2):
                            nc.tensor.matmul(
                                ps[:, ip],
                                wt[0][mh][:],
                                xps[ip][:, 0],
                                start=True,
                                stop=False,
                            )
                            nc.tensor.matmul(
                                ps[:, ip],
                                wt[1][mh][:],
                                xps[ip][:, 1],
                                start=False,
                                stop=True,
                            )
                        # ~1/6 of the PSUM->SBUF bias-add copies run on the
                        # ACT engine (activation Identity with per-partition
                        # bias), the rest on DVE, balancing both engines
                        on_act = i0 == 10
                        if g in (0, NG - 1):
                            # one of the two rows is the 0/63 boundary row,
                            # whose bias count varies per tensor: copy that
                            # row per tensor, merge the uniform row per pair
                            hv = 0 if g == 0 else 1  # varying-count row
                            hu = 1 - hv
                            for ip in range(2):
                                col = (i0 + ip) * HQ + 2 * g + hv
                                if on_act:
                                    nc.scalar.activation(
                                        obufs[mh][:, hv, i0 + ip],
                                        ps[:, ip, hv],
                                        mybir.ActivationFunctionType.Identity,
                                        bias=bvt[mh][:, col : col + 1],
                                    )
                                else:
                                    nc.vector.tensor_scalar_add(
                                        obufs[mh][:, hv, i0 + ip],
                                        ps[:, ip, hv],
                                        bvt[mh][:, col : col + 1],
                                    )
                            col = i0 * HQ + 2 * g + hu
                            if on_act:
                                nc.scalar.activation(
                                    obufs[mh][:, hu, i0 : i0 + 2],
                                    ps[:, :, hu],
                                    mybir.ActivationFunctionType.Identity,
                                    bias=bvt[mh][:, col : col + 1],
                                )
                            else:
                                nc.vector.tensor_scalar_add(
                                    obufs[mh][:, hu, i0 : i0 + 2],
                                    ps[:, :, hu],
                                    bvt[mh][:, col : col + 1],
                                )
                        else:
                            # interior rows: count uniform (12) across both
                            # tensors and rows -> one op per pair
                            col = i0 * HQ + 2 * g
                            src = ps[:].transpose([0, 2, 1, 3])  # (hl, ip, w)
                            if on_act:
                                nc.scalar.activation(
                                    obufs[mh][:, :, i0 : i0 + 2],
                                    src,
                                    mybir.ActivationFunctionType.Identity,
                                    bias=bvt[mh][:, col : col + 1],
                                )
                            else:
                                nc.vector.tensor_scalar_add(
                                    obufs[mh][:, :, i0 : i0 + 2],
                                    src,
                                    bvt[mh][:, col : col + 1],
                                )
                for mh in range(2):
                    # outputs split across the ACT HWDGE ring and the SWDGE
                    # (gpsimd) ring; small pieces keep each DMA-lane hold
                    # short to avoid head-of-line blocking
                    for q in range(6):
                        eng = (
                            nc.gpsimd
                            if (mh * 6 + q) in (1, 3, 5, 8, 10)
                            else nc.scalar
                        )
                        eng.dma_start(
                            out=y_d[
                                mh * 128 : (mh + 1) * 128,
                                24 * g + 4 * q : 24 * g + 4 * (q + 1),
                                :,
                            ],
                            in_=obufs[mh][:, q // 3, (q % 3) * 4 : (q % 3) * 4 + 4],
                        )
    nc.finalize()
    _NC_CACHE["nc"] = nc
    return nc


def _counts() -> np.ndarray:
    """count[r] for output row r (conv-transpose bias multiplicity)."""
    r = np.arange(HOUT)
    return (np.minimum(11, r) - np.maximum(0, r - (HOUT - NT)) + 1).astype(np.float32)


def shard_inputs(inputs: dict) -> list[dict]:
    xs = [np.ascontiguousarray(np.asarray(inputs[f"x{i}"], dtype=np.float32)) for i in range(NT)]
    w = np.ascontiguousarray(np.asarray(inputs["W"], dtype=np.float32))
    b = np.asarray(inputs["b"], dtype=np.float32)
    counts = _counts()
    in_maps = []
    for cid in range(NCORES):
        b_idx, hq = divmod(cid, 4)
        h0 = hq * HQ
        x_core = np.empty((NT, C, HQ, WD), dtype=np.float32)
        for i in range(NT):
            x_core[i] = xs[i][b_idx, :, h0 : h0 + HQ, :]
        # bv[mh, o, i*HQ + hl] = b[mh*128+o] * count(12*(h0+hl) + i)
        i_idx = np.arange(NT)[:, None]
        hl_idx = np.arange(HQ)[None, :]
        cnt = counts[12 * (h0 + hl_idx) + i_idx].reshape(NT * HQ)  # [192]
        bv = (b.reshape(2, 128)[:, :, None] * cnt[None, None, :]).astype(np.float32)
        in_maps.append({"x": x_core, "w": w, "bv": bv})
    return in_maps


def gather_outputs(results: list[dict]) -> np.ndarray:
    out = np.empty((B, C, HOUT, WD), dtype=np.float32)
    for cid in range(NCORES):
        b_idx, hq = divmod(cid, 4)
        h0 = hq * HQ
        out[b_idx, :, 12 * h0 : 12 * h0 + NT * HQ, :] = results[cid]["y"]
    return out


def kernel(**inputs) -> np.ndarray:
    nc = build_nc()
    in_maps = shard_inputs(inputs)
    res = run_bass_kernel_spmd(nc, in_maps, core_ids=list(range(NCORES)))
    return gather_outputs(res.results)



# revision 10
# speedup vs baseline: 1.3567x; 1.3567x over previous
"""TRN2 Bass kernel for nn_Construct_76484777607483.

Computes, for 12 input tensors x_i [B=2, C=256, H=64, W=256]:
    y_i = einsum('bchw,co->bohw', x_i, W)
interleaved over H (output row 12*h + i comes from tensor i, row h) into
out [2, 256, 768, 256], plus bias b[o] * count(row) where count is the
conv-transpose overlap multiplicity (ramp 1..12 at the top edge, 12 in the
middle, 12..1 at the bottom edge).

Sharding: 8 cores = (2 batches) x (4 h-quarters of 16 input rows). Each core
handles all 12 tensors for its 16 rows; the row-interleave is assembled
on-chip so output DMA writes are contiguous per channel.

Cost-model-driven design: only 3 DMA rings exist (SP, ACT, Pool) and ring
cost is per-partition bytes at ~2.6 GB/s/partition, so f32 I/O (303 us over
3 rings) would dominate the 82 us PE floor. Inputs and outputs are bf16
(error ~5e-3 << 2e-2 gate), halving ring work. The PSUM->SBUF drain does
double duty: f32->bf16 convert + bias add. GPSIMD cannot read PSUM on HW,
so copies split mh0->DVE / mh1->ACT; Pool carries the output-chunk DMAs.
Output drains in 2-tensor chunks throughout each 4-row unit so PSUM never
backs up; the final unit routes its tail chunks through the by-then-idle
SP ring in 2-row halves so the end-of-kernel drain is short.
  PE  384 matmuls ~82us | SP in-DMA ~78us | Pool out-DMA ~75us
  DVE copies ~60us | ACT copies ~58us    -> ~89.6us cost-model
"""

import numpy as np
import ml_dtypes

import concourse.bacc as bacc
import concourse.tile as tile
import concourse.mybir as mybir
from concourse.bass_utils import run_bass_kernel_spmd

B, C, H, WD = 2, 256, 64, 256
NT = 12                 # stacked tensors
NCORES = 8
HQ = H // 4             # 16 input rows per core
NQ = 4                  # 4-row units per core
HOUT = NT * H           # 768

_F32 = mybir.dt.float32
_BF16 = mybir.dt.bfloat16

_NC_CACHE = {}


def _emit_copy(nc, eng, dst, src, bias):
    """PSUM f32 -> SBUF bf16 copy with per-partition bias add."""
    if eng == "scalar":
        nc.scalar.activation(
            dst, src, mybir.ActivationFunctionType.Identity, bias=bias
        )
    else:
        nc.vector.tensor_scalar_add(dst, src, bias)


def build_nc():
    if "nc" in _NC_CACHE:
        return _NC_CACHE["nc"]
    nc = bacc.Bacc("TRN2", target_bir_lowering=False)
    x_d = nc.declare_dram_parameter("x", [NT, C, HQ, WD], _BF16, isOutput=False)
    w_d = nc.declare_dram_parameter("w", [2, 2, 128, 128], _BF16, isOutput=False)
    bv_d = nc.declare_dram_parameter("bv", [2, 128, NT * HQ], _F32, isOutput=False)
    # y rows factored [q, hl, i]: row 48q + 12hl + i (same linear layout as
    # [C, 192, WD]) so chunked out-DMAs slice i-ranges directly
    y_d = nc.declare_dram_parameter("y", [C, NQ, 4, NT, WD], _BF16, isOutput=True)

    with tile.TileContext(nc) as tc:
        with (
            tc.tile_pool(name="const", bufs=1) as cpool,
            tc.tile_pool(name="xin", bufs=8) as xpool,
            tc.tile_pool(name="obuf", bufs=2) as obpool,
            tc.tile_pool(name="ps", bufs=2, space="PSUM") as pspool,
        ):
            # consts off the SP ring so the first x tile loads immediately;
            # first-needed W tiles on Pool (ACT's activation-table preload
            # delays its ring)
            wt = [
                [cpool.tile([128, 128], _BF16, name=f"w{kh}{mh}") for mh in range(2)]
                for kh in range(2)
            ]
            bvt = [cpool.tile([128, NT * HQ], _F32, name=f"bv{mh}") for mh in range(2)]
            nc.gpsimd.dma_start(out=wt[0][0][:], in_=w_d[0, 0, :, :])
            nc.gpsimd.dma_start(out=wt[1][0][:], in_=w_d[1, 0, :, :])
            nc.scalar.dma_start(out=wt[0][1][:], in_=w_d[0, 1, :, :])
            nc.scalar.dma_start(out=wt[1][1][:], in_=w_d[1, 1, :, :])
            for mh in range(2):
                nc.gpsimd.dma_start(out=bvt[mh][:], in_=bv_d[mh, :, :])

            for q in range(NQ):
                # [128, hl(4), i(12), 256] bf16 interleave buffer per mh
                obufs = [
                    obpool.tile([128, 4, NT, WD], _BF16, name=f"ob{q}_{mh}",
                                tag=f"ob{mh}")
                    for mh in range(2)
                ]
                for i in range(NT):
                    # input tile [128, 4 rows, 256] bf16 per kh-half
                    xts = []
                    for kh in range(2):
                        xt = xpool.tile([128, 4, WD], _BF16,
                                        name=f"x{q}_{i}_{kh}", tag=f"x{kh}")
                        nc.sync.dma_start(
                            out=xt[:],
                            in_=x_d[i, kh * 128:(kh + 1) * 128,
                                    4 * q:4 * q + 4, :],
                        )
                        xts.append(xt)
                    for mh in range(2):
                        # [128, 4 rows, 256] f32 = 2 PSUM banks
                        ps = pspool.tile([128, 4, WD], _F32,
                                         name=f"ps{q}_{i}_{mh}", tag=f"ps{mh}")
                        for ip in range(2):
                            nc.tensor.matmul(
                                ps[:, 2 * ip:2 * ip + 2],
                                wt[0][mh][:], xts[0][:, 2 * ip:2 * ip + 2],
                                start=True, stop=False,
                            )
                            nc.tensor.matmul(
                                ps[:, 2 * ip:2 * ip + 2],
                                wt[1][mh][:], xts[1][:, 2 * ip:2 * ip + 2],
                                start=False, stop=True,
                            )
                        # drain: f32 PSUM -> bf16 obuf[:, hl, i, :] with bias
                        # b[o]*count. Interior rows share count=12 -> one op;
                        # the global edge rows (h=0 of q=0, h=15 of q=3) have
                        # per-tensor counts -> split op (SPMD program shared
                        # across cores, data differs via bv)
                        eng = "vector" if mh == 0 else "scalar"
                        col0 = i * HQ + 4 * q
                        ob = obufs[mh]
                        if q == 0:
                            _emit_copy(nc, eng, ob[:, 0, i, :], ps[:, 0],
                                       bvt[mh][:, col0:col0 + 1])
                            _emit_copy(nc, eng, ob[:, 1:4, i, :], ps[:, 1:4],
                                       bvt[mh][:, col0 + 1:col0 + 2])
                        elif q == NQ - 1:
                            _emit_copy(nc, eng, ob[:, 0:3, i, :], ps[:, 0:3],
                                       bvt[mh][:, col0:col0 + 1])
                            _emit_copy(nc, eng, ob[:, 3, i, :], ps[:, 3],
                                       bvt[mh][:, col0 + 3:col0 + 4])
                        else:
                            _emit_copy(nc, eng, ob[:, :, i, :], ps[:],
                                       bvt[mh][:, col0:col0 + 1])
                    # drain finished tensors to DRAM as we go: 2-tensor
                    # chunks [128, 4, 2, 256] bf16 on the Pool ring mid-unit,
                    # single-tensor chunks for the last two; the final unit
                    # uses the idle SP ring and 2-row halves to shorten the
                    # end-of-kernel drain
                    if i % 2 == 1 and i < 10:
                        for mh in range(2):
                            nc.gpsimd.dma_start(
                                out=y_d[mh * 128:(mh + 1) * 128, q, :,
                                        i - 1:i + 1, :],
                                in_=obufs[mh][:, :, i - 1:i + 1, :],
                            )
                    elif i >= 10:
                        last = q == NQ - 1
                        rings = (("sync", "scalar") if last
                                 else ("gpsimd", "gpsimd"))
                        if last and i == 11:
                            for mh, ring in ((0, rings[0]), (1, rings[1])):
                                for hh in range(2):
                                    rr = ring if hh == 0 else (
                                        "sync" if ring != "sync" else "gpsimd")
                                    getattr(nc, rr).dma_start(
                                        out=y_d[mh * 128:(mh + 1) * 128, q,
                                                2 * hh:2 * hh + 2, i:i + 1, :],
                                        in_=obufs[mh][:, 2 * hh:2 * hh + 2,
                                                      i:i + 1, :],
                                    )
                        else:
                            for mh, ring in ((0, rings[0]), (1, rings[1])):
                                getattr(nc, ring).dma_start(
                                    out=y_d[mh * 128:(mh + 1) * 128, q, :,
                                            i:i + 1, :],
                                    in_=obufs[mh][:, :, i:i + 1, :],
                                )
    nc.finalize()
    _NC_CACHE["nc"] = nc
    return nc


def _counts() -> np.ndarray:
    """count[r] for output row r (conv-transpose bias multiplicity)."""
    r = np.arange(HOUT)
    return (np.minimum(11, r) - np.maximum(0, r - (HOUT - NT)) + 1).astype(np.float32)


def shard_inputs(inputs: dict) -> list[dict]:
    xs = [np.asarray(inputs[f"x{i}"], dtype=np.float32) for i in range(NT)]
    w = np.asarray(inputs["W"], dtype=np.float32)
    b = np.asarray(inputs["b"], dtype=np.float32)
    # [kh, mh, k, m] bf16 weight tiles
    w_t = np.ascontiguousarray(
        w.reshape(2, 128, 2, 128).transpose(0, 2, 1, 3)
    ).astype(ml_dtypes.bfloat16)
    counts = _counts()
    in_maps = []
    for cid in range(NCORES):
        b_idx, hq = divmod(cid, 4)
        h0 = hq * HQ
        x_core = np.empty((NT, C, HQ, WD), dtype=ml_dtypes.bfloat16)
        for i in range(NT):
            x_core[i] = xs[i][b_idx, :, h0:h0 + HQ, :].astype(ml_dtypes.bfloat16)
        # bv[mh, o, i*HQ + hl] = b[mh*128+o] * count(12*(h0+hl) + i)
        i_idx = np.arange(NT)[:, None]
        hl_idx = np.arange(HQ)[None, :]
        cnt = counts[12 * (h0 + hl_idx) + i_idx].reshape(NT * HQ)  # [192]
        bv = (b.reshape(2, 128)[:, :, None] * cnt[None, None, :]).astype(np.float32)
        in_maps.append({"x": x_core, "w": w_t, "bv": bv})
    return in_maps


def gather_outputs(results: list[dict]) -> np.ndarray:
    out = np.empty((B, C, HOUT, WD), dtype=np.float32)
    for cid in range(NCORES):
        b_idx, hq = divmod(cid, 4)
        h0 = hq * HQ
        y = np.asarray(results[cid]["y"]).reshape(C, NT * HQ, WD)
        out[b_idx, :, 12 * h0:12 * h0 + NT * HQ, :] = y.astype(np.float32)
    return out


def kernel(**inputs) -> np.ndarray:
    nc = build_nc()
    in_maps = shard_inputs(inputs)
    res = run_bass_kernel_spmd(nc, in_maps, core_ids=list(range(NCORES)))
    return gather_outputs(res.results)


# revision 11
# speedup vs baseline: 1.5359x; 1.1321x over previous
"""TRN2 Bass kernel for nn_Construct_76484777607483.

Computes, for 12 input tensors x_i [B=2, C=256, H=64, W=256]:
    y_i = einsum('bchw,co->bohw', x_i, W)
interleaved over H (output row 12*h + i comes from tensor i, row h) into
out [2, 256, 768, 256], plus bias b[o] * count(row) (conv-transpose overlap
multiplicity: ramp 1..12 at the edges, 12 in the middle).

Sharding: 8 cores = (2 batches) x (4 h-quarters of 16 input rows); each core
handles all 12 tensors for its 16 rows and assembles the row-interleave
on-chip.

fp8 DoubleRow design: the PE DoubleRow perf mode contracts K=256 (two
128-planes) per matmul at 0.5 cycles/row, so a 3-pass residual scheme
    y = q(8W)^T q(x/8) + q(8W/32)^T q(32 xr) + q(8W - q(8W))^T q(x/8)
(xr = x/8 - q(x/8), all quantized fp8e4m3) computes the full-precision
product in 768 PE-cycles per [128,512] tile instead of bf16's 1024, with
rel err ~2.6e-3 (same as bf16; gate is 2e-2). Inputs ship as packed
x8+xr fp8 pairs (2B/elem, same ring cost as bf16). PE drops to ~62us and
the three DMA rings + the two PSUM-drain engines become the constraint:
  ACT copies(mh1)+2 chunk-cols ~73us | SP x-in ~70us | Pool chunks ~69us
  PE ~62us | DVE copies(mh0) ~60us   -> ~79.2us cost-model
Output drains in 2-tensor bf16 chunks throughout each 4-row unit; the final
unit routes tail chunks through the idle SP ring.
"""

import numpy as np
import ml_dtypes

import concourse.bacc as bacc
import concourse.tile as tile
import concourse.mybir as mybir
from concourse.bass_utils import run_bass_kernel_spmd

B, C, H, WD = 2, 256, 64, 256
NT = 12                 # stacked tensors
NCORES = 8
HQ = H // 4             # 16 input rows per core
NQ = 4                  # 4-row units per core
HOUT = NT * H           # 768

_F32 = mybir.dt.float32
_BF16 = mybir.dt.bfloat16
_E4 = mybir.dt.float8e4
_E4NP = mybir.dt.np(_E4)
_DR = mybir.MatmulPerfMode.DoubleRow

_SW, _SX, _S2 = 8.0, 1.0 / 8.0, 32.0

_NC_CACHE = {}


def _emit_copy(nc, eng, dst, src, bias):
    """PSUM f32 -> SBUF bf16 copy with per-partition bias add."""
    if eng == "scalar":
        nc.scalar.activation(
            dst, src, mybir.ActivationFunctionType.Identity, bias=bias
        )
    else:
        nc.vector.tensor_scalar_add(dst, src, bias)


def build_nc():
    if "nc" in _NC_CACHE:
        return _NC_CACHE["nc"]
    nc = bacc.Bacc("TRN2", target_bir_lowering=False)
    # x packed [tensor, partition, var(x8|xr), plane(k-half), row, w] fp8
    x_d = nc.declare_dram_parameter("x", [NT, 128, 2, 2, HQ, WD], _E4, isOutput=False)
    # w [pass(W8|W8b|Wr8), mh, partition, plane, m] fp8
    w_d = nc.declare_dram_parameter("w", [3, 2, 128, 2, 128], _E4, isOutput=False)
    bv_d = nc.declare_dram_parameter("bv", [2, 128, NT * HQ], _F32, isOutput=False)
    # y rows factored [q, hl, i]: row 48q + 12hl + i (same linear layout as
    # [C, 192, WD]) so chunked out-DMAs slice i-ranges directly
    y_d = nc.declare_dram_parameter("y", [C, NQ, 4, NT, WD], _BF16, isOutput=True)

    with tile.TileContext(nc) as tc:
        with (
            tc.tile_pool(name="const", bufs=1) as cpool,
            tc.tile_pool(name="xin", bufs=8) as xpool,
            tc.tile_pool(name="xh", bufs=2) as xhpool,
            tc.tile_pool(name="x0", bufs=1) as x0pool,
            tc.tile_pool(name="obuf", bufs=2) as obpool,
            tc.tile_pool(name="ps", bufs=2, space="PSUM") as pspool,
        ):
            wt = [
                [cpool.tile([128, 2, 128], _E4, name=f"w{p_}{mh}") for mh in range(2)]
                for p_ in range(3)
            ]
            bvt = [cpool.tile([128, NT * HQ], _F32, name=f"bv{mh}") for mh in range(2)]
            # pass-1 W on Pool (first needed); pass-2/3 W on ACT (its
            # activation-table preload delays the ring anyway)
            nc.gpsimd.dma_start(out=wt[0][0][:], in_=w_d[0, 0, :, :, :])
            nc.gpsimd.dma_start(out=wt[0][1][:], in_=w_d[0, 1, :, :, :])
            for p_ in (1, 2):
                for mh in range(2):
                    nc.scalar.dma_start(out=wt[p_][mh][:], in_=w_d[p_, mh, :, :, :])

            xtile = {}

            def load_x(q, i, ring, pool, tag):
                xt = pool.tile([128, 2, 2, 4, WD], _E4, name=f"x{q}_{i}", tag=tag)
                getattr(nc, ring).dma_start(
                    out=xt[:], in_=x_d[i, :, :, :, 4 * q:4 * q + 4, :]
                )
                xtile[(q, i)] = xt

            # kernel-head: x(0,0) in 2 row-halves on SP (earlier first
            # matmul); x(0,1)/x(0,2) on the otherwise-idle Pool/ACT heads
            t0 = x0pool.tile([128, 2, 2, 4, WD], _E4, name="x00s")
            for hh in range(2):
                nc.sync.dma_start(
                    out=t0[:, :, :, 2 * hh:2 * hh + 2, :],
                    in_=x_d[0, :, :, :, 2 * hh:2 * hh + 2, :],
                )
            xtile[(0, 0)] = t0
            load_x(0, 1, "gpsimd", x0pool, "xp01")
            load_x(0, 2, "scalar", x0pool, "xa02")
            for mh in range(2):
                nc.gpsimd.dma_start(out=bvt[mh][:], in_=bv_d[mh, :, :])

            for q in range(NQ):
                obufs = [
                    obpool.tile([128, 4, NT, WD], _BF16, name=f"ob{q}_{mh}",
                                tag=f"ob{mh}")
                    for mh in range(2)
                ]
                for i in range(NT):
                    if (q, i) not in xtile:
                        load_x(q, i, "sync", xpool, "x")
                    xt = xtile.pop((q, i))
                    for mh in range(2):
                        ps = pspool.tile([128, 4, WD], _F32,
                                         name=f"ps{q}_{i}_{mh}", tag=f"ps{mh}")
                        for ip in range(2):
                            o = ps[:, 2 * ip:2 * ip + 2]
                            m0 = xt[:, 0, :, 2 * ip:2 * ip + 2]
                            m1 = xt[:, 1, :, 2 * ip:2 * ip + 2]
                            nc.tensor.matmul(o, wt[0][mh][:], m0,
                                             start=True, stop=False, perf_mode=_DR)
                            nc.tensor.matmul(o, wt[1][mh][:], m1,
                                             start=False, stop=False, perf_mode=_DR)
                            nc.tensor.matmul(o, wt[2][mh][:], m0,
                                             start=False, stop=True, perf_mode=_DR)
                        # drain: f32 PSUM -> bf16 obuf[:, hl, i, :] with bias
                        # b[o]*count; interior rows share count=12 -> one op;
                        # the global edge rows (h=0 of q=0, h=15 of q=3) have
                        # per-tensor counts -> split op (SPMD program shared,
                        # data differs via bv)
                        eng = "vector" if mh == 0 else "scalar"
                        col0 = i * HQ + 4 * q
                        ob = obufs[mh]
                        if q == 0:
                            _emit_copy(nc, eng, ob[:, 0, i, :], ps[:, 0],
                                       bvt[mh][:, col0:col0 + 1])
                            _emit_copy(nc, eng, ob[:, 1:4, i, :], ps[:, 1:4],
                                       bvt[mh][:, col0 + 1:col0 + 2])
                        elif q == NQ - 1:
                            _emit_copy(nc, eng, ob[:, 0:3, i, :], ps[:, 0:3],
                                       bvt[mh][:, col0:col0 + 1])
                            _emit_copy(nc, eng, ob[:, 3, i, :], ps[:, 3],
                                       bvt[mh][:, col0 + 3:col0 + 4])
                        else:
                            _emit_copy(nc, eng, ob[:, :, i, :], ps[:],
                                       bvt[mh][:, col0:col0 + 1])
                    # output chunks: 2-tensor bf16 chunks as tensors finish;
                    # chunk columns 1,3 put mh0 on ACT (Pool would otherwise
                    # bind); the q+1 first tensor prefetches at the Pool
                    # queue head opened after chunk col 4
                    if i % 2 == 1 and i < 10:
                        k = i // 2
                        r0 = nc.scalar if k in (1, 3) else nc.gpsimd
                        for mh, ring in ((0, r0), (1, nc.gpsimd)):
                            ring.dma_start(
                                out=y_d[mh * 128:(mh + 1) * 128, q, :,
                                        i - 1:i + 1, :],
                                in_=obufs[mh][:, :, i - 1:i + 1, :],
                            )
                        if i == 9 and q < NQ - 1:
                            load_x(q + 1, 0, "gpsimd", xhpool, "xh0")
                    elif i >= 10:
                        last = q == NQ - 1
                        rings = (("sync", "scalar") if last
                                 else ("gpsimd", "gpsimd"))
                        if last and i == 11:
                            for mh, ring in ((0, rings[0]), (1, rings[1])):
                                for hh in range(2):
                                    rr = ring if hh == 0 else (
                                        "sync" if ring != "sync" else "gpsimd")
                                    getattr(nc, rr).dma_start(
                                        out=y_d[mh * 128:(mh + 1) * 128, q,
                                                2 * hh:2 * hh + 2, i:i + 1, :],
                                        in_=obufs[mh][:, 2 * hh:2 * hh + 2,
                                                      i:i + 1, :],
                                    )
                        else:
                            for mh, ring in ((0, rings[0]), (1, rings[1])):
                                getattr(nc, ring).dma_start(
                                    out=y_d[mh * 128:(mh + 1) * 128, q, :,
                                            i:i + 1, :],
                                    in_=obufs[mh][:, :, i:i + 1, :],
                                )
    nc.finalize()
    _NC_CACHE["nc"] = nc
    return nc


def _counts() -> np.ndarray:
    """count[r] for output row r (conv-transpose bias multiplicity)."""
    r = np.arange(HOUT)
    return (np.minimum(11, r) - np.maximum(0, r - (HOUT - NT)) + 1).astype(np.float32)


def _q8(v: np.ndarray) -> np.ndarray:
    return v.astype(_E4NP).astype(np.float32)


def shard_inputs(inputs: dict) -> list[dict]:
    xs = [np.asarray(inputs[f"x{i}"], dtype=np.float32) for i in range(NT)]
    w = np.asarray(inputs["W"], dtype=np.float32)
    b = np.asarray(inputs["b"], dtype=np.float32)
    # 3-pass fp8 weight planes [pass, mh, p, plane, m]
    w8 = _q8(w * _SW)
    w8b = _q8(w * _SW / _S2)
    wr8 = _q8(w * _SW - w8)
    w_t = np.empty((3, 2, 128, 2, 128), dtype=_E4NP)
    for p_, wf in enumerate((w8, w8b, wr8)):
        # wf[k_in, m_out]; plane pl covers k in [pl*128, (pl+1)*128)
        w_t[p_] = np.ascontiguousarray(
            wf.reshape(2, 128, 2, 128).transpose(2, 1, 0, 3)
        ).astype(_E4NP)
    counts = _counts()
    in_maps = []
    for cid in range(NCORES):
        b_idx, hq = divmod(cid, 4)
        h0 = hq * HQ
        x_core = np.empty((NT, 128, 2, 2, HQ, WD), dtype=_E4NP)
        for i in range(NT):
            xf = xs[i][b_idx, :, h0:h0 + HQ, :] * _SX        # [256, 16, 256]
            x8 = _q8(xf)
            xr8 = ((xf - x8) * _S2).astype(_E4NP)
            x8 = x8.astype(_E4NP)
            # [c, h, w] -> [p, var, plane, h, w] with c = plane*128 + p
            x_core[i, :, 0] = x8.reshape(2, 128, HQ, WD).transpose(1, 0, 2, 3)
            x_core[i, :, 1] = xr8.reshape(2, 128, HQ, WD).transpose(1, 0, 2, 3)
        # bv[mh, o, i*HQ + hl] = b[mh*128+o] * count(12*(h0+hl) + i)
        i_idx = np.arange(NT)[:, None]
        hl_idx = np.arange(HQ)[None, :]
        cnt = counts[12 * (h0 + hl_idx) + i_idx].reshape(NT * HQ)  # [192]
        bv = (b.reshape(2, 128)[:, :, None] * cnt[None, None, :]).astype(np.float32)
        in_maps.append({"x": x_core, "w": w_t, "bv": bv})
    return in_maps


def gather_outputs(results: list[dict]) -> np.ndarray:
    out = np.empty((B, C, HOUT, WD), dtype=np.float32)
    for cid in range(NCORES):
        b_idx, hq = divmod(cid, 4)
        h0 = hq * HQ
        y = np.asarray(results[cid]["y"]).reshape(C, NT * HQ, WD)
        out[b_idx, :, 12 * h0:12 * h0 + NT * HQ, :] = y.astype(np.float32)
    return out


def kernel(**inputs) -> np.ndarray:
    nc = build_nc()
    in_maps = shard_inputs(inputs)
    res = run_bass_kernel_spmd(nc, in_maps, core_ids=list(range(NCORES)))
    return gather_outputs(res.results)
